# revision 4
# baseline (speedup 1.0000x reference)
"""Trainium2 Bass kernel for nn_AttentionConvHead (windowed per-channel attention).

Math (per batch b, all channels d independent):
    Q = Wq @ q + bq ; K = Wk @ k + bk ; V = Wv @ v + bv        (1x1 convs)
    out[d,t,n] = sum_i softmax_i(Q[d,t,n] * Kpad[d,t+i,n]) * Vpad[d,t+i,n]
with K/V zero-padded by 3 on the time axis (pad contributes exp(0)=1 to the
softmax denominator and 0 to the numerator).

Distribution: pure data-parallel, one batch element per NeuronCore (B=8).

Per-core layout: partitions p = c + 64*g pack (channel, n-half) where the
n axis (207, padded to 208) is split into two groups of 104. Free dim is
(t outer, n_local inner), so a time-shift by i is a contiguous free-dim
offset of i*104. Projections are 128x128x(F) matmuls with block-diagonal
bf16 weights; window sums accumulate in fp32 PSUM via bf16 identity
matmuls; exp runs on ScalarE; score/value products on VectorE (bf16, 2x
mode) with two score muls per window offloaded to GpSimd.
"""

import numpy as np

B, C, T, N = 8, 64, 128, 207
D = 64
KS, PAD = 7, 3
NPAD, NG, P = 208, 104, 128
F = T * NG                 # 13312 free positions per partition
TP = T + 2 * PAD           # 134 padded time steps
FPAD = TP * NG             # 13936
MM = 512                   # psum bank = 512 fp32 matmul columns
XCH = 2048                 # phase-A input DMA chunk
CHUNKS = [1536] * 8 + [1024]   # phase-B chunks (sum = F)
N_GP = 2                   # score muls per window on GpSimd

_CACHE = {}


def _build():
    import concourse.bacc as bacc
    import concourse.mybir as mybir
    from concourse.tile import TileContext

    f32 = mybir.dt.float32
    bf16 = mybir.dt.bfloat16
    AF = mybir.ActivationFunctionType

    nc = bacc.Bacc("TRN2", target_bir_lowering=False)

    xq = nc.declare_dram_parameter("xq", [P, F], bf16, isOutput=False)
    xk = nc.declare_dram_parameter("xk", [P, F], bf16, isOutput=False)
    xv = nc.declare_dram_parameter("xv", [P, F], bf16, isOutput=False)
    wq = nc.declare_dram_parameter("wq", [P, P], bf16, isOutput=False)
    wk = nc.declare_dram_parameter("wk", [P, P], bf16, isOutput=False)
    wv = nc.declare_dram_parameter("wv", [P, P], bf16, isOutput=False)
    bqd = nc.declare_dram_parameter("bq", [P, 1], f32, isOutput=False)
    bkd = nc.declare_dram_parameter("bk", [P, 1], f32, isOutput=False)
    bvd = nc.declare_dram_parameter("bv", [P, 1], f32, isOutput=False)
    idd = nc.declare_dram_parameter("ident", [P, P], bf16, isOutput=False)
    out_d = nc.declare_dram_parameter("out", [P, F], f32, isOutput=True)

    from contextlib import ExitStack

    with TileContext(nc) as tc, ExitStack() as ctx:
        consts = ctx.enter_context(tc.tile_pool(name="consts", bufs=1))
        xin = ctx.enter_context(tc.tile_pool(name="xin", bufs=4))
        big = ctx.enter_context(tc.tile_pool(name="big", bufs=1))
        work = ctx.enter_context(tc.tile_pool(name="work", bufs=3))
        outp = ctx.enter_context(tc.tile_pool(name="outp", bufs=3))
        psA = ctx.enter_context(tc.tile_pool(name="psA", bufs=2, space="PSUM"))
        psB = ctx.enter_context(tc.tile_pool(name="psB", bufs=1, space="PSUM"))

        wq_s = consts.tile([P, P], bf16, tag="wq")
        wk_s = consts.tile([P, P], bf16, tag="wk")
        wv_s = consts.tile([P, P], bf16, tag="wv")
        id_s = consts.tile([P, P], bf16, tag="ident")
        bq_s = consts.tile([P, 1], f32, tag="bq")
        bk_s = consts.tile([P, 1], f32, tag="bk")
        bv_s = consts.tile([P, 1], f32, tag="bv")
        nc.sync.dma_start(out=wq_s, in_=wq.ap())
        nc.sync.dma_start(out=wk_s, in_=wk.ap())
        nc.sync.dma_start(out=wv_s, in_=wv.ap())
        nc.sync.dma_start(out=id_s, in_=idd.ap())
        nc.sync.dma_start(out=bq_s, in_=bqd.ap())
        nc.sync.dma_start(out=bk_s, in_=bkd.ap())
        nc.sync.dma_start(out=bv_s, in_=bvd.ap())

        xq_s = big.tile([P, F], bf16, tag="xq")
        Kp = big.tile([P, FPAD], bf16, tag="Kp")
        Vp = big.tile([P, FPAD], bf16, tag="Vp")

        nc.sync.dma_start(out=xq_s, in_=xq.ap())
        nc.vector.memset(Kp[:, 0 : PAD * NG], 0.0)
        nc.vector.memset(Kp[:, FPAD - PAD * NG : FPAD], 0.0)
        nc.vector.memset(Vp[:, 0 : PAD * NG], 0.0)
        nc.vector.memset(Vp[:, FPAD - PAD * NG : FPAD], 0.0)

        # Phase A: project K and V, evict (+bias, ->bf16) into padded tiles.
        for j0 in range(0, F, XCH):
            ch = min(XCH, F - j0)
            kt = xin.tile([P, XCH], bf16, tag="xin")
            nc.sync.dma_start(out=kt[:, :ch], in_=xk.ap()[:, j0 : j0 + ch])
            vt = xin.tile([P, XCH], bf16, tag="xin")
            nc.sync.dma_start(out=vt[:, :ch], in_=xv.ap()[:, j0 : j0 + ch])
            for m0 in range(0, ch, MM):
                psk = psA.tile([P, MM], f32, tag="psA")
                nc.tensor.matmul(psk, wk_s, kt[:, m0 : m0 + MM], start=True, stop=True)
                nc.scalar.activation(
                    Kp[:, PAD * NG + j0 + m0 : PAD * NG + j0 + m0 + MM],
                    psk,
                    AF.Identity,
                    bias=bk_s,
                    scale=1.0,
                )
                psv = psA.tile([P, MM], f32, tag="psA")
                nc.tensor.matmul(psv, wv_s, vt[:, m0 : m0 + MM], start=True, stop=True)
                nc.vector.tensor_scalar_add(
                    Vp[:, PAD * NG + j0 + m0 : PAD * NG + j0 + m0 + MM],
                    psv,
                    bv_s,
                )

        # Phase B: per chunk, project Q then windowed softmax-attention.
        base = 0
        for fc in CHUNKS:
            qb = work.tile([P, fc], bf16, tag="qb")
            for m0 in range(0, fc, MM):
                qp = psA.tile([P, MM], f32, tag="psA")
                nc.tensor.matmul(
                    qp, wq_s, xq_s[:, base + m0 : base + m0 + MM], start=True, stop=True
                )
                nc.scalar.activation(
                    qb[:, m0 : m0 + MM], qp, AF.Identity, bias=bq_s, scale=1.0
                )

            den = psB.tile([P, 1536], f32, tag="den", bufs=1)
            num = psB.tile([P, 1536], f32, tag="num", bufs=1)
            for i in range(KS):
                koff = base + i * NG
                sb = work.tile([P, fc], bf16, tag="sb")
                if i < KS - N_GP:
                    nc.vector.tensor_mul(sb, qb, Kp[:, koff : koff + fc])
                else:
                    nc.gpsimd.tensor_mul(sb, qb, Kp[:, koff : koff + fc])
                eb = work.tile([P, fc], bf16, tag="eb")
                nc.scalar.activation(eb, sb, AF.Exp)
                wb = work.tile([P, fc], bf16, tag="wb")
                nc.vector.tensor_mul(wb, eb, Vp[:, koff : koff + fc])
                first, last = i == 0, i == KS - 1
                for m0 in range(0, fc, MM):
                    nc.tensor.matmul(
                        den[:, m0 : m0 + MM],
                        id_s,
                        eb[:, m0 : m0 + MM],
                        start=first,
                        stop=last,
                        skip_group_check=True,
                    )
                    nc.tensor.matmul(
                        num[:, m0 : m0 + MM],
                        id_s,
                        wb[:, m0 : m0 + MM],
                        start=first,
                        stop=last,
                        skip_group_check=True,
                    )
            r = work.tile([P, fc], f32, tag="r")
            nc.vector.reciprocal_approx_fast(out=r, in_=den[:, :fc])
            ot = outp.tile([P, fc], f32, tag="ot")
            nc.vector.tensor_mul(ot, num[:, :fc], r)
            nc.sync.dma_start(out=out_d.ap()[:, base : base + fc], in_=ot)
            base += fc

    nc.compile()
    return nc


def _pack(x, bf):
    """[C, T, N] f32 -> [128, F] bf16: partition p = c + 64g, free = t*104 + n_loc."""
    xp = np.zeros((C, T, NPAD), np.float32)
    xp[:, :, :N] = x
    return np.ascontiguousarray(
        xp.reshape(C, T, 2, NG).transpose(2, 0, 1, 3).reshape(P, F)
    ).astype(bf)


def _unpack(o):
    """[128, F] -> [D, T, N]."""
    return np.ascontiguousarray(
        o.reshape(2, D, T, NG).transpose(1, 2, 0, 3).reshape(D, T, NPAD)[:, :, :N]
    )


def _lhsT_blockdiag(W):
    Z = np.zeros((P, P), np.float32)
    Z[:C, :D] = W.T
    Z[C:, D:] = W.T
    return Z


def _prep_in_maps(q, k, v, Wq, bq, Wk, bk, Wv, bv):
    import ml_dtypes

    bf = ml_dtypes.bfloat16
    wqp = _lhsT_blockdiag(np.asarray(Wq, np.float32)).astype(bf)
    wkp = _lhsT_blockdiag(np.asarray(Wk, np.float32)).astype(bf)
    wvp = _lhsT_blockdiag(np.asarray(Wv, np.float32)).astype(bf)
    bqp = np.concatenate([bq, bq]).reshape(P, 1).astype(np.float32)
    bkp = np.concatenate([bk, bk]).reshape(P, 1).astype(np.float32)
    bvp = np.concatenate([bv, bv]).reshape(P, 1).astype(np.float32)
    ident = np.eye(P, dtype=np.float32).astype(bf)
    in_maps = []
    for b in range(B):
        in_maps.append(
            {
                "xq": _pack(np.asarray(q[b], np.float32), bf),
                "xk": _pack(np.asarray(k[b], np.float32), bf),
                "xv": _pack(np.asarray(v[b], np.float32), bf),
                "wq": wqp,
                "wk": wkp,
                "wv": wvp,
                "bq": bqp,
                "bk": bkp,
                "bv": bvp,
                "ident": ident,
            }
        )
    return in_maps


def run(inputs, trace=False):
    """Build (cached), run on 8 cores, return (output, BassKernelResults)."""
    from concourse.bass_utils import run_bass_kernel_spmd

    if "nc" not in _CACHE:
        _CACHE["nc"] = _build()
    nc = _CACHE["nc"]
    in_maps = _prep_in_maps(**inputs)
    res = run_bass_kernel_spmd(nc, in_maps, core_ids=list(range(B)), trace=trace)
    out = np.stack([_unpack(np.asarray(res.results[b]["out"])) for b in range(B)])
    return out, res


def kernel(q, k, v, Wq, bq, Wk, bk, Wv, bv):
    out, _ = run(dict(q=q, k=k, v=v, Wq=Wq, bq=bq, Wk=Wk, bk=bk, Wv=Wv, bv=bv))
    return out


# revision 5
# speedup vs baseline: 1.4161x; 1.4161x over previous
"""Trainium2 Bass kernel for nn_AttentionConvHead (windowed per-channel attention).

Math (per batch b, all channels d independent):
    Q = Wq @ q + bq ; K = Wk @ k + bk ; V = Wv @ v + bv        (1x1 convs)
    out[d,t,n] = sum_i softmax_i(Q[d,t,n] * Kpad[d,t+i,n]) * Vpad[d,t+i,n]
with K/V zero-padded by 3 on the time axis (pad contributes exp(0)=1 to the
softmax denominator and 0 to the numerator).

Distribution: pure data-parallel, one batch element per NeuronCore (B=8).

Per-core layout: partitions p = c + 64*g pack (channel, n-half) where the
n axis (207, padded to 208) is split into two groups of 104. Free dim is
(t outer, n_local inner), so a time-shift by i is a contiguous free-dim
offset of i*104. Projections are 128x128x(F) matmuls with block-diagonal
bf16 weights; window sums accumulate in fp32 PSUM via bf16 identity
matmuls; exp runs on ScalarE; score/value products on VectorE (bf16, 2x
mode) with two score muls per window offloaded to GpSimd.
"""

import numpy as np

B, C, T, N = 8, 64, 128, 207
D = 64
KS, PAD = 7, 3
NPAD, NG, P = 208, 104, 128
F = T * NG                 # 13312 free positions per partition
TP = T + 2 * PAD           # 134 padded time steps
FPAD = TP * NG             # 13936
MM = 512                   # psum bank = 512 fp32 matmul columns
XCH = 2048                 # phase-A input DMA chunk
CHUNKS = [1536] * 8 + [1024]   # phase-B chunks (sum = F)
N_GP = 0                   # score muls per window on GpSimd (contends with DVE port)

_CACHE = {}


def _build():
    import concourse.bacc as bacc
    import concourse.mybir as mybir
    from concourse.tile import TileContext

    f32 = mybir.dt.float32
    bf16 = mybir.dt.bfloat16
    AF = mybir.ActivationFunctionType

    nc = bacc.Bacc("TRN2", target_bir_lowering=False)

    xq = nc.declare_dram_parameter("xq", [P, F], bf16, isOutput=False)
    xk = nc.declare_dram_parameter("xk", [P, F], bf16, isOutput=False)
    xv = nc.declare_dram_parameter("xv", [P, F], bf16, isOutput=False)
    wq = nc.declare_dram_parameter("wq", [P, P], bf16, isOutput=False)
    wk = nc.declare_dram_parameter("wk", [P, P], bf16, isOutput=False)
    wv = nc.declare_dram_parameter("wv", [P, P], bf16, isOutput=False)
    bqd = nc.declare_dram_parameter("bq", [P, 1], f32, isOutput=False)
    bkd = nc.declare_dram_parameter("bk", [P, 1], f32, isOutput=False)
    bvd = nc.declare_dram_parameter("bv", [P, 1], f32, isOutput=False)
    idd = nc.declare_dram_parameter("ident", [P, P], bf16, isOutput=False)
    out_d = nc.declare_dram_parameter("out", [P, F], f32, isOutput=True)

    from contextlib import ExitStack

    with TileContext(nc) as tc, ExitStack() as ctx:
        consts = ctx.enter_context(tc.tile_pool(name="consts", bufs=1))
        xin = ctx.enter_context(tc.tile_pool(name="xin", bufs=4))
        big = ctx.enter_context(tc.tile_pool(name="big", bufs=1))
        work = ctx.enter_context(tc.tile_pool(name="work", bufs=3))
        outp = ctx.enter_context(tc.tile_pool(name="outp", bufs=3))
        psA = ctx.enter_context(tc.tile_pool(name="psA", bufs=2, space="PSUM"))
        psB = ctx.enter_context(tc.tile_pool(name="psB", bufs=1, space="PSUM"))

        wq_s = consts.tile([P, P], bf16, tag="wq")
        wk_s = consts.tile([P, P], bf16, tag="wk")
        wv_s = consts.tile([P, P], bf16, tag="wv")
        id_s = consts.tile([P, P], bf16, tag="ident")
        bq_s = consts.tile([P, 1], f32, tag="bq")
        bk_s = consts.tile([P, 1], f32, tag="bk")
        bv_s = consts.tile([P, 1], f32, tag="bv")
        nc.sync.dma_start(out=wq_s, in_=wq.ap())
        nc.sync.dma_start(out=wk_s, in_=wk.ap())
        nc.sync.dma_start(out=wv_s, in_=wv.ap())
        nc.sync.dma_start(out=id_s, in_=idd.ap())
        nc.sync.dma_start(out=bq_s, in_=bqd.ap())
        nc.sync.dma_start(out=bk_s, in_=bkd.ap())
        nc.sync.dma_start(out=bv_s, in_=bvd.ap())

        xq_s = big.tile([P, F], bf16, tag="xq")
        Kp = big.tile([P, FPAD], bf16, tag="Kp")
        Vp = big.tile([P, FPAD], bf16, tag="Vp")

        nc.sync.dma_start(out=xq_s, in_=xq.ap())
        nc.vector.memset(Kp[:, 0 : PAD * NG], 0.0)
        nc.vector.memset(Kp[:, FPAD - PAD * NG : FPAD], 0.0)
        nc.vector.memset(Vp[:, 0 : PAD * NG], 0.0)
        nc.vector.memset(Vp[:, FPAD - PAD * NG : FPAD], 0.0)

        # Phase A: project K and V, evict (+bias, ->bf16) into padded tiles.
        for j0 in range(0, F, XCH):
            ch = min(XCH, F - j0)
            kt = xin.tile([P, XCH], bf16, tag="xin")
            nc.sync.dma_start(out=kt[:, :ch], in_=xk.ap()[:, j0 : j0 + ch])
            vt = xin.tile([P, XCH], bf16, tag="xin")
            nc.sync.dma_start(out=vt[:, :ch], in_=xv.ap()[:, j0 : j0 + ch])
            for m0 in range(0, ch, MM):
                psk = psA.tile([P, MM], f32, tag="psA")
                nc.tensor.matmul(psk, wk_s, kt[:, m0 : m0 + MM], start=True, stop=True)
                nc.scalar.activation(
                    Kp[:, PAD * NG + j0 + m0 : PAD * NG + j0 + m0 + MM],
                    psk,
                    AF.Identity,
                    bias=bk_s,
                    scale=1.0,
                )
                psv = psA.tile([P, MM], f32, tag="psA")
                nc.tensor.matmul(psv, wv_s, vt[:, m0 : m0 + MM], start=True, stop=True)
                nc.vector.tensor_scalar_add(
                    Vp[:, PAD * NG + j0 + m0 : PAD * NG + j0 + m0 + MM],
                    psv,
                    bv_s,
                )

        # Phase B: per chunk, project Q then windowed softmax-attention.
        base = 0
        for fc in CHUNKS:
            qb = work.tile([P, fc], bf16, tag="qb")
            for m0 in range(0, fc, MM):
                qp = psA.tile([P, MM], f32, tag="psA")
                nc.tensor.matmul(
                    qp, wq_s, xq_s[:, base + m0 : base + m0 + MM], start=True, stop=True
                )
                nc.scalar.activation(
                    qb[:, m0 : m0 + MM], qp, AF.Identity, bias=bq_s, scale=1.0
                )

            den = psB.tile([P, 1536], f32, tag="den", bufs=1)
            num = psB.tile([P, 1536], f32, tag="num", bufs=1)
            for i in range(KS):
                koff = base + i * NG
                sb = work.tile([P, fc], bf16, tag="sb")
                if i < KS - N_GP:
                    nc.vector.tensor_mul(sb, qb, Kp[:, koff : koff + fc])
                else:
                    nc.gpsimd.tensor_mul(sb, qb, Kp[:, koff : koff + fc])
                eb = work.tile([P, fc], bf16, tag="eb")
                nc.scalar.activation(eb, sb, AF.Exp)
                wb = work.tile([P, fc], bf16, tag="wb")
                nc.vector.tensor_mul(wb, eb, Vp[:, koff : koff + fc])
                first, last = i == 0, i == KS - 1
                for m0 in range(0, fc, MM):
                    nc.tensor.matmul(
                        den[:, m0 : m0 + MM],
                        id_s,
                        eb[:, m0 : m0 + MM],
                        start=first,
                        stop=last,
                        skip_group_check=True,
                    )
                    nc.tensor.matmul(
                        num[:, m0 : m0 + MM],
                        id_s,
                        wb[:, m0 : m0 + MM],
                        start=first,
                        stop=last,
                        skip_group_check=True,
                    )
            r = work.tile([P, fc], f32, tag="r")
            nc.vector.reciprocal_approx_fast(out=r, in_=den[:, :fc])
            ot = outp.tile([P, fc], f32, tag="ot")
            nc.vector.tensor_mul(ot, num[:, :fc], r)
            nc.sync.dma_start(out=out_d.ap()[:, base : base + fc], in_=ot)
            base += fc

    nc.compile()
    return nc


def _pack(x, bf):
    """[C, T, N] f32 -> [128, F] bf16: partition p = c + 64g, free = t*104 + n_loc."""
    xp = np.zeros((C, T, NPAD), np.float32)
    xp[:, :, :N] = x
    return np.ascontiguousarray(
        xp.reshape(C, T, 2, NG).transpose(2, 0, 1, 3).reshape(P, F)
    ).astype(bf)


def _unpack(o):
    """[128, F] -> [D, T, N]."""
    return np.ascontiguousarray(
        o.reshape(2, D, T, NG).transpose(1, 2, 0, 3).reshape(D, T, NPAD)[:, :, :N]
    )


def _lhsT_blockdiag(W):
    Z = np.zeros((P, P), np.float32)
    Z[:C, :D] = W.T
    Z[C:, D:] = W.T
    return Z


def _prep_in_maps(q, k, v, Wq, bq, Wk, bk, Wv, bv):
    import ml_dtypes

    bf = ml_dtypes.bfloat16
    wqp = _lhsT_blockdiag(np.asarray(Wq, np.float32)).astype(bf)
    wkp = _lhsT_blockdiag(np.asarray(Wk, np.float32)).astype(bf)
    wvp = _lhsT_blockdiag(np.asarray(Wv, np.float32)).astype(bf)
    bqp = np.concatenate([bq, bq]).reshape(P, 1).astype(np.float32)
    bkp = np.concatenate([bk, bk]).reshape(P, 1).astype(np.float32)
    bvp = np.concatenate([bv, bv]).reshape(P, 1).astype(np.float32)
    ident = np.eye(P, dtype=np.float32).astype(bf)
    in_maps = []
    for b in range(B):
        in_maps.append(
            {
                "xq": _pack(np.asarray(q[b], np.float32), bf),
                "xk": _pack(np.asarray(k[b], np.float32), bf),
                "xv": _pack(np.asarray(v[b], np.float32), bf),
                "wq": wqp,
                "wk": wkp,
                "wv": wvp,
                "bq": bqp,
                "bk": bkp,
                "bv": bvp,
                "ident": ident,
            }
        )
    return in_maps


def run(inputs, trace=False):
    """Build (cached), run on 8 cores, return (output, BassKernelResults)."""
    from concourse.bass_utils import run_bass_kernel_spmd

    if "nc" not in _CACHE:
        _CACHE["nc"] = _build()
    nc = _CACHE["nc"]
    in_maps = _prep_in_maps(**inputs)
    res = run_bass_kernel_spmd(nc, in_maps, core_ids=list(range(B)), trace=trace)
    out = np.stack([_unpack(np.asarray(res.results[b]["out"])) for b in range(B)])
    return out, res


def kernel(q, k, v, Wq, bq, Wk, bk, Wv, bv):
    out, _ = run(dict(q=q, k=k, v=v, Wq=Wq, bq=bq, Wk=Wk, bk=bk, Wv=Wv, bv=bv))
    return out


# revision 6
# speedup vs baseline: 1.5813x; 1.1167x over previous
"""Trainium2 Bass kernel for nn_AttentionConvHead (windowed per-channel attention).

Math (per batch b, all channels d independent):
    Q = Wq @ q + bq ; K = Wk @ k + bk ; V = Wv @ v + bv        (1x1 convs)
    out[d,t,n] = sum_i softmax_i(Q[d,t,n] * Kpad[d,t+i,n]) * Vpad[d,t+i,n]
with K/V zero-padded by 3 on the time axis (pad contributes exp(0)=1 to the
softmax denominator and 0 to the numerator).

Distribution: pure data-parallel, one batch element per NeuronCore (B=8).

Per-core layout: partitions p = c + 64*g pack (channel, n-half); n (207,
padded to 208) splits into two groups of 104. Free dim is (t outer, n_local
inner) so a time shift is a contiguous free-dim offset of i*104.
Projections: 128x128 block-diagonal bf16 matmuls. Window sums: fp32 PSUM
accumulation via bf16 identity matmuls. exp + all PSUM->SBUF evictions on
ScalarE; score/value products (bf16 2x) + reciprocal + final mul on VectorE.
Phase A (K/V proj) and phase B (attention) are emission-interleaved so the
per-engine instruction streams pipeline across phases.
"""

import numpy as np

B, C, T, N = 8, 64, 128, 207
D = 64
KS, PAD = 7, 3
NPAD, NG, P = 208, 104, 128
F = T * NG                 # 13312 free positions per partition
TP = T + 2 * PAD           # 134 padded time steps
FPAD = TP * NG             # 13936
MM = 512                   # psum bank = 512 fp32 matmul columns
XCH = 4096                 # phase-A input DMA chunk
CHUNKS = [1536] * 8 + [1024]   # phase-B chunks (sum = F)

_CACHE = {}


def _build():
    import concourse.bacc as bacc
    import concourse.mybir as mybir
    from concourse.tile import TileContext

    f32 = mybir.dt.float32
    bf16 = mybir.dt.bfloat16
    AF = mybir.ActivationFunctionType

    nc = bacc.Bacc("TRN2", target_bir_lowering=False)

    xq = nc.declare_dram_parameter("xq", [P, F], bf16, isOutput=False)
    xk = nc.declare_dram_parameter("xk", [P, F], bf16, isOutput=False)
    xv = nc.declare_dram_parameter("xv", [P, F], bf16, isOutput=False)
    wq = nc.declare_dram_parameter("wq", [P, P], bf16, isOutput=False)
    wk = nc.declare_dram_parameter("wk", [P, P], bf16, isOutput=False)
    wv = nc.declare_dram_parameter("wv", [P, P], bf16, isOutput=False)
    bqd = nc.declare_dram_parameter("bq", [P, 1], f32, isOutput=False)
    bkd = nc.declare_dram_parameter("bk", [P, 1], f32, isOutput=False)
    bvd = nc.declare_dram_parameter("bv", [P, 1], f32, isOutput=False)
    idd = nc.declare_dram_parameter("ident", [P, P], bf16, isOutput=False)
    out_d = nc.declare_dram_parameter("out", [P, F], f32, isOutput=True)

    from contextlib import ExitStack

    with TileContext(nc) as tc, ExitStack() as ctx:
        consts = ctx.enter_context(tc.tile_pool(name="consts", bufs=1))
        xin = ctx.enter_context(tc.tile_pool(name="xin", bufs=4))
        big = ctx.enter_context(tc.tile_pool(name="big", bufs=1))
        work = ctx.enter_context(tc.tile_pool(name="work", bufs=3))
        outp = ctx.enter_context(tc.tile_pool(name="outp", bufs=3))
        psA = ctx.enter_context(tc.tile_pool(name="psA", bufs=2, space="PSUM"))
        psB = ctx.enter_context(tc.tile_pool(name="psB", bufs=1, space="PSUM"))

        wq_s = consts.tile([P, P], bf16, tag="wq")
        wk_s = consts.tile([P, P], bf16, tag="wk")
        wv_s = consts.tile([P, P], bf16, tag="wv")
        id_s = consts.tile([P, P], bf16, tag="ident")
        bq_s = consts.tile([P, 1], f32, tag="bq")
        bk_s = consts.tile([P, 1], f32, tag="bk")
        bv_s = consts.tile([P, 1], f32, tag="bv")
        nc.sync.dma_start(out=wq_s, in_=wq.ap())
        nc.sync.dma_start(out=wk_s, in_=wk.ap())
        nc.sync.dma_start(out=wv_s, in_=wv.ap())
        nc.sync.dma_start(out=id_s, in_=idd.ap())
        nc.sync.dma_start(out=bq_s, in_=bqd.ap())
        nc.sync.dma_start(out=bk_s, in_=bkd.ap())
        nc.sync.dma_start(out=bv_s, in_=bvd.ap())

        xq_s = big.tile([P, F], bf16, tag="xq")
        Kp = big.tile([P, FPAD], bf16, tag="Kp")
        Vp = big.tile([P, FPAD], bf16, tag="Vp")

        nc.vector.memset(Kp[:, 0 : PAD * NG], 0.0)
        nc.vector.memset(Kp[:, FPAD - PAD * NG : FPAD], 0.0)
        nc.vector.memset(Vp[:, 0 : PAD * NG], 0.0)
        nc.vector.memset(Vp[:, FPAD - PAD * NG : FPAD], 0.0)

        def emit_A(j0):
            """DMA + project + evict one XCH chunk of K and V (and stream xq in)."""
            ch = min(XCH, F - j0)
            nc.sync.dma_start(out=xq_s[:, j0 : j0 + ch], in_=xq.ap()[:, j0 : j0 + ch])
            kt = xin.tile([P, XCH], bf16, tag="xin")
            nc.sync.dma_start(out=kt[:, :ch], in_=xk.ap()[:, j0 : j0 + ch])
            vt = xin.tile([P, XCH], bf16, tag="xin")
            nc.sync.dma_start(out=vt[:, :ch], in_=xv.ap()[:, j0 : j0 + ch])
            for m0 in range(0, ch, MM):
                psk = psA.tile([P, MM], f32, tag="psA")
                nc.tensor.matmul(psk, wk_s, kt[:, m0 : m0 + MM], start=True, stop=True)
                nc.scalar.activation(
                    Kp[:, PAD * NG + j0 + m0 : PAD * NG + j0 + m0 + MM],
                    psk,
                    AF.Identity,
                    bias=bk_s,
                    scale=1.0,
                )
                psv = psA.tile([P, MM], f32, tag="psA")
                nc.tensor.matmul(psv, wv_s, vt[:, m0 : m0 + MM], start=True, stop=True)
                nc.scalar.activation(
                    Vp[:, PAD * NG + j0 + m0 : PAD * NG + j0 + m0 + MM],
                    psv,
                    AF.Identity,
                    bias=bv_s,
                    scale=1.0,
                )

        def emit_B(base, fc):
            """One phase-B chunk: Q proj + 7-window softmax attention."""
            qb = work.tile([P, fc], bf16, tag="qb")
            for m0 in range(0, fc, MM):
                qp = psA.tile([P, MM], f32, tag="psA")
                nc.tensor.matmul(
                    qp, wq_s, xq_s[:, base + m0 : base + m0 + MM], start=True, stop=True
                )
                nc.scalar.activation(
                    qb[:, m0 : m0 + MM], qp, AF.Identity, bias=bq_s, scale=1.0
                )
            den = psB.tile([P, 1536], f32, tag="den", bufs=1)
            num = psB.tile([P, 1536], f32, tag="num", bufs=1)
            for i in range(KS):
                koff = base + i * NG
                sb = work.tile([P, fc], bf16, tag="sb")
                nc.vector.tensor_mul(sb, qb, Kp[:, koff : koff + fc])
                eb = work.tile([P, fc], bf16, tag="eb")
                nc.scalar.activation(eb, sb, AF.Exp)
                wb = work.tile([P, fc], bf16, tag="wb")
                nc.vector.tensor_mul(wb, eb, Vp[:, koff : koff + fc])
                first, last = i == 0, i == KS - 1
                for m0 in range(0, fc, MM):
                    nc.tensor.matmul(
                        den[:, m0 : m0 + MM],
                        id_s,
                        eb[:, m0 : m0 + MM],
                        start=first,
                        stop=last,
                        skip_group_check=True,
                    )
                    nc.tensor.matmul(
                        num[:, m0 : m0 + MM],
                        id_s,
                        wb[:, m0 : m0 + MM],
                        start=first,
                        stop=last,
                        skip_group_check=True,
                    )
            r = work.tile([P, fc], f32, tag="r")
            nc.vector.reciprocal_approx_fast(out=r, in_=den[:, :fc])
            ot = outp.tile([P, fc], f32, tag="ot")
            nc.vector.tensor_mul(ot, num[:, :fc], r)
            nc.sync.dma_start(out=out_d.ap()[:, base : base + fc], in_=ot)

        # Interleave phase A and B so per-engine streams pipeline: emit a B
        # chunk as soon as the A coverage its windows need has been emitted.
        a_steps = list(range(0, F, XCH))
        b_chunks = []
        base = 0
        for fc in CHUNKS:
            b_chunks.append((base, fc))
            base += fc
        ai = 0
        for base, fc in b_chunks:
            need = min(F, base + fc + PAD * NG)  # interior coverage needed
            while ai < len(a_steps) and a_steps[ai] < need:
                emit_A(a_steps[ai])
                ai += 1
            emit_B(base, fc)
        while ai < len(a_steps):
            emit_A(a_steps[ai])
            ai += 1

    nc.compile()
    return nc


def _pack(x, bf):
    """[C, T, N] f32 -> [128, F] bf16: partition p = c + 64g, free = t*104 + n_loc."""
    xp = np.zeros((C, T, NPAD), np.float32)
    xp[:, :, :N] = x
    return np.ascontiguousarray(
        xp.reshape(C, T, 2, NG).transpose(2, 0, 1, 3).reshape(P, F)
    ).astype(bf)


def _unpack(o):
    """[128, F] -> [D, T, N]."""
    return np.ascontiguousarray(
        o.reshape(2, D, T, NG).transpose(1, 2, 0, 3).reshape(D, T, NPAD)[:, :, :N]
    )


def _lhsT_blockdiag(W):
    Z = np.zeros((P, P), np.float32)
    Z[:C, :D] = W.T
    Z[C:, D:] = W.T
    return Z


def _prep_in_maps(q, k, v, Wq, bq, Wk, bk, Wv, bv):
    import ml_dtypes

    bf = ml_dtypes.bfloat16
    wqp = _lhsT_blockdiag(np.asarray(Wq, np.float32)).astype(bf)
    wkp = _lhsT_blockdiag(np.asarray(Wk, np.float32)).astype(bf)
    wvp = _lhsT_blockdiag(np.asarray(Wv, np.float32)).astype(bf)
    bqp = np.concatenate([bq, bq]).reshape(P, 1).astype(np.float32)
    bkp = np.concatenate([bk, bk]).reshape(P, 1).astype(np.float32)
    bvp = np.concatenate([bv, bv]).reshape(P, 1).astype(np.float32)
    ident = np.eye(P, dtype=np.float32).astype(bf)
    in_maps = []
    for b in range(B):
        in_maps.append(
            {
                "xq": _pack(np.asarray(q[b], np.float32), bf),
                "xk": _pack(np.asarray(k[b], np.float32), bf),
                "xv": _pack(np.asarray(v[b], np.float32), bf),
                "wq": wqp,
                "wk": wkp,
                "wv": wvp,
                "bq": bqp,
                "bk": bkp,
                "bv": bvp,
                "ident": ident,
            }
        )
    return in_maps


def run(inputs, trace=False):
    """Build (cached), run on 8 cores, return (output, BassKernelResults)."""
    from concourse.bass_utils import run_bass_kernel_spmd

    if "nc" not in _CACHE:
        _CACHE["nc"] = _build()
    nc = _CACHE["nc"]
    in_maps = _prep_in_maps(**inputs)
    res = run_bass_kernel_spmd(nc, in_maps, core_ids=list(range(B)), trace=trace)
    out = np.stack([_unpack(np.asarray(res.results[b]["out"])) for b in range(B)])
    return out, res


def kernel(q, k, v, Wq, bq, Wk, bk, Wv, bv):
    out, _ = run(dict(q=q, k=k, v=v, Wq=Wq, bq=bq, Wk=Wk, bk=bk, Wv=Wv, bv=bv))
    return out
